# revision 11
# baseline (speedup 1.0000x reference)
"""Trainium2 Bass kernel for the GNN message-passing decoder (v2).

Model (per batch b):
  h0 = x @ W_lin + b_lin            -> [N=256, L2=64] per b
  h  = h0 @ W_in + b_in             -> [N, H=32]
  3 rounds of fully-connected message passing:
    rcv = h @ We1[:H], snd = h @ We1[H:]
    e1  = lrelu(rcv_i + snd_j + be1)          [N,N,HE=128]
    e2  = lrelu(e1 @ We2 + be2)               [N,N,EO=64]
    m_i = sum_j e2                            [N,EO]
    n   = lrelu([h|m] @ Wn0 + bn0); h = lrelu(n @ Wn1 + bn1)
  out = tanh(h)                      -> [B, N, H]

Strategy (v3): data parallel over batch (2 per core). Per (b, round):
feature-on-partition layout. With the identity
  relu(snd + b_i) + 0.25*snd = max(1.25*snd, 0.25*snd - b_i)
one DVE scalar_tensor_tensor per receiver yields m'' whose M matmul
P = 0.8*We2^T m'' already contains the per-sender linear term
0.2*We2^T snd, so no Q init stream is needed. The per-receiver constant
c' = We2^T b_i + be2 enters PSUM via one full-width K=32 indicator
matmul per bank (lhsT rows = on-chip c' vectors), so the Prelu needs no
bias -> one full double-bank ACT instruction. Receivers are packed
low(0:128)/high(128:256) on PSUM partition halves; M matmuls are M=64
even/odd-half pairs on separate PE tiles. Messages come from DVE
X-reduces or ACT accumulators (balance knob).
"""

import os
import sys

import numpy as np

for _p in ("/opt/trn_rl_repo", "/opt/pypackages"):
    if _p not in sys.path and os.path.isdir(_p):
        sys.path.append(_p)

# Problem dims (hardcoded per spec)
B, N, L, H, HE, EO = 16, 256, 64, 32, 128, 64
NT = 3           # message passing rounds
NCORES = 8
BPC = B // NCORES  # batches per core = 2
NP2 = N // 2
NDB = N // 8     # 32 double-banks, 8 receivers each (4 low + 4 high)

# node permutation: even nodes first (stage-1 produces this order)
PERM = np.concatenate([np.arange(0, N, 2), np.arange(1, N, 2)])
INVPERM = np.argsort(PERM)

# double-banks whose messages are reduced on DVE (else ACT accumulators)
DVE_REDUCE_EVERY = 2

_CACHE = {}


def _build_nc():
    import concourse.bass as bass
    import concourse.tile as tile
    from concourse import bacc, mybir
    from contextlib import ExitStack

    F16 = mybir.dt.float16
    F32 = mybir.dt.float32
    AF = mybir.ActivationFunctionType
    ALU = mybir.AluOpType

    nc = bacc.Bacc("TRN2", target_bir_lowering=False, debug=False)

    # ---- kernel I/O (per-core) ----
    xT_d = nc.dram_tensor("xT16", [L, BPC], F16, kind="ExternalInput")
    wlin_d = nc.dram_tensor("Wlin16", [L, N * L], F16, kind="ExternalInput")
    blT_d = nc.dram_tensor("blT16", [L, N], F16, kind="ExternalInput")  # perm'd
    win_d = nc.dram_tensor("Win16", [L, H], F16, kind="ExternalInput")
    binc_d = nc.dram_tensor("binc", [H, 1], F32, kind="ExternalInput")
    we1a_d = nc.dram_tensor("We1a16", [H, NT * HE], F16, kind="ExternalInput")
    we1b_d = nc.dram_tensor("We1b16", [H, NT * HE], F16, kind="ExternalInput")
    pbe1_d = nc.dram_tensor("pbe1", [HE, NT], F32, kind="ExternalInput")
    w8_d = nc.dram_tensor("w8c16", [HE, NT * EO], F16, kind="ExternalInput")
    wm_d = nc.dram_tensor("wm32", [HE, NT * EO], F32, kind="ExternalInput")
    be2r_d = nc.dram_tensor("be2r", [1, NT * EO], F32, kind="ExternalInput")
    ones_d = nc.dram_tensor("ones1", [1, N], F32, kind="ExternalInput")
    ind2_d = nc.dram_tensor("ind32", [128, 16 * 512], F16, kind="ExternalInput")
    wn0_d = nc.dram_tensor("Wn0c16", [H + EO, NT * H], F16, kind="ExternalInput")
    bn0_d = nc.dram_tensor("bn0c", [H, NT], F32, kind="ExternalInput")
    wn1_d = nc.dram_tensor("Wn1c16", [H, NT * H], F16, kind="ExternalInput")
    bn1_d = nc.dram_tensor("bn1c", [H, NT], F32, kind="ExternalInput")
    out_d = nc.dram_tensor("out", [BPC, H, N], F32, kind="ExternalOutput")

    with tile.TileContext(nc) as tc, ExitStack() as ctx:
        const = ctx.enter_context(tc.tile_pool(name="const", bufs=1))
        perb = ctx.enter_context(tc.tile_pool(name="perb", bufs=2))
        mpool = ctx.enter_context(tc.tile_pool(name="m", bufs=12))
        e2pool = ctx.enter_context(tc.tile_pool(name="e2p", bufs=3))
        small = ctx.enter_context(tc.tile_pool(name="small", bufs=4))
        psum = ctx.enter_context(tc.tile_pool(name="psum", bufs=3, space="PSUM"))
        ppsum = ctx.enter_context(tc.tile_pool(name="ppsum", bufs=2, space="PSUM"))

        # ---- load constants ----
        def load(dram, shape, dt):
            t = const.tile(shape, dt, tag=dram.name)
            nc.sync.dma_start(t[:, :], dram[:, :])
            return t

        xTs = load(xT_d, [L, BPC], F16)
        blT = load(blT_d, [L, N], F16)
        win = load(win_d, [L, H], F16)
        binc = load(binc_d, [H, 1], F32)
        we1a = load(we1a_d, [H, NT * HE], F16)
        we1b = load(we1b_d, [H, NT * HE], F16)
        pbe1 = load(pbe1_d, [HE, NT], F32)
        w8 = load(w8_d, [HE, NT * EO], F16)
        wm = load(wm_d, [HE, NT * EO], F32)
        be2r = load(be2r_d, [1, NT * EO], F32)
        ones1 = load(ones_d, [1, N], F32)
        ind32 = load(ind2_d, [128, 16 * 512], F16)
        wn0 = load(wn0_d, [H + EO, NT * H], F16)
        bn0 = load(bn0_d, [H, NT], F32)
        wn1 = load(wn1_d, [H, NT * H], F16)
        bn1 = load(bn1_d, [H, NT], F32)

        wlin = const.tile([L, N * L], F16, tag="wlin")
        for k in range(4):
            sl = bass.ts(k, N * L // 4)
            eng = nc.sync if k % 2 == 0 else nc.gpsimd
            eng.dma_start(wlin[:, sl], wlin_d[:, sl])

        # ---- stage 1: h0 = x @ W_lin (transposed, perm'd via A/B split) ----
        h0p = ppsum.tile([128, 2 * NP2], F32, tag="prep")
        for np_ in range(NP2):
            lhsT = wlin[:, np_ * 2 * L:(np_ + 1) * 2 * L]
            nc.tensor.matmul(h0p[:, 2 * np_:2 * np_ + 2], lhsT, xTs[:, :],
                             start=True, stop=True, skip_group_check=True)
        hstA = const.tile([L, 2 * NP2], F16, tag="hstA")  # even nodes
        hstB = const.tile([L, 2 * NP2], F16, tag="hstB")  # odd nodes
        nc.scalar.copy(hstA[:, :], h0p[0:L, :])
        nc.scalar.copy(hstB[:, :], h0p[L:2 * L, :])
        hsvA = hstA[:, :].rearrange("p (n two) -> p two n", two=2)
        hsvB = hstB[:, :].rearrange("p (n two) -> p two n", two=2)

        hT = []  # per-b hidden state [H, N] fp16 (perm'd node order)
        for b in range(BPC):
            htp = ppsum.tile([H, N], F32, tag="prep")
            nc.tensor.matmul(htp[:, :], win[:, :], blT[:, :],
                             start=True, stop=False, skip_group_check=True)
            nc.tensor.matmul(htp[:, 0:NP2], win[:, :], hsvA[:, b:b + 1, :],
                             start=False, stop=True, skip_group_check=True)
            nc.tensor.matmul(htp[:, NP2:N], win[:, :], hsvB[:, b:b + 1, :],
                             start=False, stop=True, skip_group_check=True)
            ht = perb.tile([H, N], F16, tag=f"hT{b}")
            nc.scalar.activation(ht[:, :], htp[:, :], AF.Identity,
                                 bias=binc[:, 0:1])
            hT.append(ht)

        # ---- rounds ----
        for t in range(NT):
            w8t = w8[:, bass.ts(t, EO)]
            wmt = wm[:, bass.ts(t, EO)]

            stage = {}
            for b in range(BPC):
                ht = hT[b]
                # receivers: posb = rcv + be1  [HE, N] fp32
                rcvp = ppsum.tile([HE, N], F32, tag="prep")
                nc.tensor.matmul(rcvp[:, :], we1a[:, bass.ts(t, HE)], ht[:, :],
                                 start=True, stop=True, skip_group_check=True)
                posb = perb.tile([HE, N], F32, tag="posb")
                nc.scalar.activation(posb[:, :], rcvp[:, :], AF.Identity,
                                     bias=pbe1[:, t:t + 1])

                # c' rows: cT[i, :] = We2^T b_i + be2   (recv-on-partition;
                # cols 0:EO = low receivers, EO:2EO = high)
                ctp = ppsum.tile([HE, 2 * EO], F32, tag="prep")
                for g in range(2):
                    cs = slice(g * EO, (g + 1) * EO)
                    nc.tensor.matmul(ctp[:, cs], posb[:, bass.ts(g, 128)],
                                     wmt, start=True, stop=False,
                                     skip_group_check=True)
                    nc.tensor.matmul(ctp[:, cs], ones1[0:1, 0:128],
                                     be2r[0:1, bass.ts(t, EO)],
                                     start=False, stop=True,
                                     skip_group_check=True)
                cTb = perb.tile([HE, 2 * EO], F16, tag="cTb")
                nc.scalar.copy(cTb[:, :], ctp[:, :])

                # senders: s125 = 1.25*snd, s025 = 0.25*snd fp16
                sndp = ppsum.tile([HE, N], F32, tag="prep")
                nc.tensor.matmul(sndp[:, :], we1b[:, bass.ts(t, HE)], ht[:, :],
                                 start=True, stop=True, skip_group_check=True)
                s125 = perb.tile([HE, N], F16, tag="s125")
                nc.scalar.activation(s125[:, :], sndp[:, :], AF.Identity,
                                     scale=1.25)
                s025 = perb.tile([HE, N], F16, tag="s025")
                nc.scalar.activation(s025[:, :], sndp[:, :], AF.Identity,
                                     scale=0.25)

                mr = perb.tile([HE, NP2], F16, tag=f"mr{b}")
                stage[b] = (posb, cTb, s125, s025, mr)

            # ---- edge loop: interleave batches per double-bank ----
            for k in range(NDB):
                for b in range(BPC):
                    posb, cTb, s125, s025, mr = stage[b]
                    # m'' = max(1.25*snd, 0.25*snd - b_r) per receiver
                    ars = []
                    for g in range(2):
                        alow = mpool.tile([HE, 512], F16, tag="mlo")
                        ahigh = mpool.tile([HE, 512], F16, tag="mhi")
                        for j in range(2):
                            rl = 4 * k + 2 * g + j
                            rh = 128 + rl
                            cs = slice(j * 256, (j + 1) * 256)
                            nc.vector.scalar_tensor_tensor(
                                alow[:, cs], s025[:, :], posb[:, rl:rl + 1],
                                s125[:, :], ALU.subtract, ALU.max)
                            nc.vector.scalar_tensor_tensor(
                                ahigh[:, cs], s025[:, :], posb[:, rh:rh + 1],
                                s125[:, :], ALU.subtract, ALU.max)
                        ars.append((alow, ahigh))

                    pbs = psum.tile([HE, 1024], F32, tag="pb",
                                    name=f"pb_{t}_{b}_{k}")
                    for g in range(2):
                        alow, ahigh = ars[g]
                        cs = slice(g * 512, (g + 1) * 512)
                        r0 = 4 * k + 2 * g
                        a32 = (r0 // 32) * 32
                        v = (r0 % 32) // 2
                        iv = ind32[a32:a32 + 32, v * 512:(v + 1) * 512]
                        nc.tensor.matmul(pbs[:, cs],
                                         cTb[a32:a32 + 32, :], iv,
                                         start=True, stop=False,
                                         skip_group_check=True,
                                         tile_position=(a32, 0))
                        nc.tensor.matmul(pbs[0:EO, cs], w8t, alow[:, :],
                                         start=False, stop=True,
                                         skip_group_check=True)
                        nc.tensor.matmul(pbs[EO:HE, cs], w8t, ahigh[:, :],
                                         start=False, stop=True,
                                         skip_group_check=True)

                    if k % DVE_REDUCE_EVERY == 0:
                        # Prelu full double-bank -> fp16 arena; DVE X-reduce
                        e2 = e2pool.tile([HE, 1024], F16, tag="e2",
                                         name=f"e2_{t}_{b}_{k}")
                        nc.scalar.activation(e2[:, :], pbs[:, :], AF.Prelu,
                                             alpha=0.2)
                        e2v = e2[:, :].rearrange("p (four n) -> p four n",
                                                 four=4)
                        with nc.allow_low_precision("msg sums fp16 ok"):
                            nc.vector.tensor_reduce(
                                mr[:, 4 * k:4 * k + 4], e2v[:, :, :],
                                axis=mybir.AxisListType.X, op=ALU.add)
                    else:
                        # ACT path: in-place Prelu + accumulator sums
                        with nc.allow_low_precision("msg sums fp16 ok"):
                            for c in range(4):
                                cs = slice(c * N, (c + 1) * N)
                                nc.scalar.activation(
                                    pbs[:, cs], pbs[:, cs], AF.Prelu,
                                    alpha=0.2,
                                    accum_out=mr[:, 4 * k + c:4 * k + c + 1])

            # ---- node MLP ----
            for b in range(BPC):
                _, _, _, _, mr = stage[b]
                ht = hT[b]
                nT = perb.tile([H + EO, N], F16, tag="nT")
                nc.vector.tensor_copy(nT[0:EO, 0:NP2], mr[0:EO, :])
                nc.vector.tensor_copy(nT[0:EO, NP2:N], mr[EO:HE, :])
                nc.scalar.copy(nT[EO:EO + H, :], ht[:, :])

                n1p = ppsum.tile([H, N], F32, tag="prep")
                nc.tensor.matmul(n1p[:, :], wn0[:, bass.ts(t, H)], nT[:, :],
                                 start=True, stop=True, skip_group_check=True)
                a1 = small.tile([H, N], F16, tag="a1")
                nc.scalar.activation(a1[:, :], n1p[:, :], AF.Prelu, alpha=0.2,
                                     bias=bn0[:, t:t + 1])
                n2p = ppsum.tile([H, N], F32, tag="prep")
                nc.tensor.matmul(n2p[:, :], wn1[:, bass.ts(t, H)], a1[:, :],
                                 start=True, stop=True, skip_group_check=True)
                if t < NT - 1:
                    ht2 = perb.tile([H, N], F16, tag=f"hT{b}")
                    nc.scalar.activation(ht2[:, :], n2p[:, :], AF.Prelu,
                                         alpha=0.2, bias=bn1[:, t:t + 1])
                    hT[b] = ht2
                else:
                    hfin = small.tile([H, N], F32, tag="hfin")
                    nc.scalar.activation(hfin[:, :], n2p[:, :], AF.Prelu,
                                         alpha=0.2, bias=bn1[:, t:t + 1])
                    outT = small.tile([H, N], F32, tag="outT")
                    nc.scalar.activation(outT[:, :], hfin[:, :], AF.Tanh)
                    nc.sync.dma_start(out_d[b, :, :], outT[:, :])

    nc.compile()
    return nc


def _prepare_in_maps(inputs):
    f32 = lambda a: np.ascontiguousarray(np.asarray(a), dtype=np.float32)
    f16c = lambda a: np.ascontiguousarray(np.asarray(a, dtype=np.float32)
                                          .astype(np.float16))
    x = f32(inputs["x"])
    W_lin = f32(inputs["W_lin"])
    b_lin = f32(inputs["b_lin"])
    W_in = f32(inputs["W_in"])
    b_in = f32(inputs["b_in"])
    We1 = f32(inputs["We1"])
    be1 = f32(inputs["be1"])
    We2 = f32(inputs["We2"])
    be2 = f32(inputs["be2"])
    Wn0 = f32(inputs["Wn0"])
    bn0 = f32(inputs["bn0"])
    Wn1 = f32(inputs["Wn1"])
    bn1 = f32(inputs["bn1"])

    blT = b_lin.reshape(N, L).T                  # [L, N]
    blT_perm = np.ascontiguousarray(blT[:, PERM])
    # 16 indicator variants: variant v activates local rows 2v, 2v+1;
    # replicated at all four 32-partition offsets (fmap must start at the
    # same partition as the weights)
    ind32 = np.zeros((128, 16 * 512), np.float32)
    for v in range(16):
        for a in range(4):
            ind32[32 * a + 2 * v, v * 512:v * 512 + 256] = 1.0
            ind32[32 * a + 2 * v + 1, v * 512 + 256:(v + 1) * 512] = 1.0
    shared = {
        "Wlin16": f16c(W_lin),
        "blT16": f16c(blT_perm),
        "Win16": f16c(W_in),
        "binc": np.ascontiguousarray(b_in[:, None]),
        "We1a16": f16c(We1[:, :H, :].transpose(1, 0, 2).reshape(H, NT * HE)),
        "We1b16": f16c(We1[:, H:, :].transpose(1, 0, 2).reshape(H, NT * HE)),
        "pbe1": np.ascontiguousarray(be1.T),
        "w8c16": f16c(0.8 * We2.transpose(1, 0, 2).reshape(HE, NT * EO)),
        "wm32": np.ascontiguousarray(
            We2.transpose(1, 0, 2).reshape(HE, NT * EO)),
        "be2r": np.ascontiguousarray(be2.reshape(1, NT * EO)),
        "ones1": np.ones((1, N), np.float32),
        "ind32": ind32.astype(np.float16),
        # n is assembled as [m; h] on-chip, so reorder Wn0's input rows
        "Wn0c16": f16c(np.concatenate([Wn0[:, H:, :], Wn0[:, :H, :]], axis=1)
                       .transpose(1, 0, 2).reshape(H + EO, NT * H)),
        "bn0c": np.ascontiguousarray(bn0.T),
        "Wn1c16": f16c(Wn1.transpose(1, 0, 2).reshape(H, NT * H)),
        "bn1c": np.ascontiguousarray(bn1.T),
    }
    in_maps = []
    for c in range(NCORES):
        m = dict(shared)
        m["xT16"] = f16c(x[c * BPC:(c + 1) * BPC, :].T)
        in_maps.append(m)
    return in_maps


def _run(inputs, trace=False):
    from concourse import bass_utils

    if "nc" not in _CACHE:
        _CACHE["nc"] = _build_nc()
    nc = _CACHE["nc"]
    in_maps = _prepare_in_maps(inputs)
    res = bass_utils.run_bass_kernel_spmd(
        nc, in_maps, core_ids=list(range(NCORES)), trace=trace)
    outs = np.concatenate([r["out"] for r in res.results], axis=0)  # [B,H,N]
    out = outs[:, :, INVPERM]            # undo node permutation
    out = np.ascontiguousarray(out.transpose(0, 2, 1))  # [B, N, H]
    return out.astype(np.float32), res


def kernel(**inputs):
    out, _ = _run(inputs, trace=False)
    return out


# revision 14
# speedup vs baseline: 1.5511x; 1.5511x over previous
"""Trainium2 Bass kernel for the GNN message-passing decoder (v2).

Model (per batch b):
  h0 = x @ W_lin + b_lin            -> [N=256, L2=64] per b
  h  = h0 @ W_in + b_in             -> [N, H=32]
  3 rounds of fully-connected message passing:
    rcv = h @ We1[:H], snd = h @ We1[H:]
    e1  = lrelu(rcv_i + snd_j + be1)          [N,N,HE=128]
    e2  = lrelu(e1 @ We2 + be2)               [N,N,EO=64]
    m_i = sum_j e2                            [N,EO]
    n   = lrelu([h|m] @ Wn0 + bn0); h = lrelu(n @ Wn1 + bn1)
  out = tanh(h)                      -> [B, N, H]

Strategy (v4): data parallel over batch (2 per core). Per (b, round):
feature-on-partition layout. Edge algebra: with b_i = rcv_i + be1,
  e2_pre = Q + c' + 0.8*We2^T relu(snd + b_i),
  Q  = 0.2*We2^T snd      (streamed once per PSUM bank, full-width wdd)
  c' = We2^T b_i + be2    (per-receiver constant)
The c' term is folded into the M matmul via the pseudo-inverse
G = 1.25*We2*inv(We2^T We2) (so 0.8*We2^T G = I): with
delta_i = G*c'_i, the DVE op per receiver is a single dual-ptr-scalar
  m'_i = max(snd + (b_i+delta_i), delta_i) = relu(snd + b_i) + delta_i
and P = 0.8*We2^T m' = 0.8*We2^T relu + c'. The Prelu then needs no
bias -> one full double-bank ACT instruction (or, on the accumulator
path, bias-free in-place slice Prelus whose accum_out yields the
messages). delta/b+delta come from two 256-col matmuls per (b,t).
Receivers are packed low(0:128)/high(128:256) on PSUM partition halves;
M matmuls are M=64 even/odd-half pairs. DVE_REDUCE_EVERY picks which
double-banks reduce on DVE vs ACT accumulators (engine balance).
"""

import os
import sys

import numpy as np

for _p in ("/opt/trn_rl_repo", "/opt/pypackages"):
    if _p not in sys.path and os.path.isdir(_p):
        sys.path.append(_p)

# Problem dims (hardcoded per spec)
B, N, L, H, HE, EO = 16, 256, 64, 32, 128, 64
NT = 3           # message passing rounds
NCORES = 8
BPC = B // NCORES  # batches per core = 2
NP2 = N // 2
NDB = N // 8     # 32 double-banks, 8 receivers each (4 low + 4 high)

# node permutation: even nodes first (stage-1 produces this order)
PERM = np.concatenate([np.arange(0, N, 2), np.arange(1, N, 2)])
INVPERM = np.argsort(PERM)

# double-banks whose messages are reduced on DVE (else ACT accumulators)
DVE_REDUCE_EVERY = 4

_CACHE = {}


def _build_nc():
    import concourse.bass as bass
    import concourse.tile as tile
    from concourse import bacc, mybir
    from contextlib import ExitStack

    F16 = mybir.dt.float16
    F32 = mybir.dt.float32
    AF = mybir.ActivationFunctionType
    ALU = mybir.AluOpType

    nc = bacc.Bacc("TRN2", target_bir_lowering=False, debug=False)

    # ---- kernel I/O (per-core) ----
    xT_d = nc.dram_tensor("xT16", [L, BPC], F16, kind="ExternalInput")
    wlin_d = nc.dram_tensor("Wlin16", [L, N * L], F16, kind="ExternalInput")
    blT_d = nc.dram_tensor("blT16", [L, N], F16, kind="ExternalInput")  # perm'd
    win_d = nc.dram_tensor("Win16", [L, H], F16, kind="ExternalInput")
    binc_d = nc.dram_tensor("binc", [H, 1], F32, kind="ExternalInput")
    we1a_d = nc.dram_tensor("We1a16", [H, NT * HE], F16, kind="ExternalInput")
    we1b_d = nc.dram_tensor("We1b16", [H, NT * HE], F16, kind="ExternalInput")
    pbe1_d = nc.dram_tensor("pbe1", [HE, NT], F32, kind="ExternalInput")
    w8_d = nc.dram_tensor("w8c16", [HE, NT * EO], F16, kind="ExternalInput")
    wdd_d = nc.dram_tensor("wdd16", [HE, NT * HE], F16, kind="ExternalInput")
    apt_d = nc.dram_tensor("ApT16", [HE, NT * HE], F16, kind="ExternalInput")
    ait_d = nc.dram_tensor("AIT16", [HE, NT * HE], F16, kind="ExternalInput")
    d0_d = nc.dram_tensor("d0c", [HE, NT], F32, kind="ExternalInput")
    wn0_d = nc.dram_tensor("Wn0c16", [H + EO, NT * H], F16, kind="ExternalInput")
    bn0_d = nc.dram_tensor("bn0c", [H, NT], F32, kind="ExternalInput")
    wn1_d = nc.dram_tensor("Wn1c16", [H, NT * H], F16, kind="ExternalInput")
    bn1_d = nc.dram_tensor("bn1c", [H, NT], F32, kind="ExternalInput")
    out_d = nc.dram_tensor("out", [BPC, H, N], F32, kind="ExternalOutput")

    with tile.TileContext(nc) as tc, ExitStack() as ctx:
        const = ctx.enter_context(tc.tile_pool(name="const", bufs=1))
        perb = ctx.enter_context(tc.tile_pool(name="perb", bufs=2))
        mpool = ctx.enter_context(tc.tile_pool(name="m", bufs=12))
        e2pool = ctx.enter_context(tc.tile_pool(name="e2p", bufs=3))
        small = ctx.enter_context(tc.tile_pool(name="small", bufs=4))
        psum = ctx.enter_context(tc.tile_pool(name="psum", bufs=3, space="PSUM"))
        ppsum = ctx.enter_context(tc.tile_pool(name="ppsum", bufs=2, space="PSUM"))

        # ---- load constants ----
        def load(dram, shape, dt):
            t = const.tile(shape, dt, tag=dram.name)
            nc.sync.dma_start(t[:, :], dram[:, :])
            return t

        xTs = load(xT_d, [L, BPC], F16)
        blT = load(blT_d, [L, N], F16)
        win = load(win_d, [L, H], F16)
        binc = load(binc_d, [H, 1], F32)
        we1a = load(we1a_d, [H, NT * HE], F16)
        we1b = load(we1b_d, [H, NT * HE], F16)
        pbe1 = load(pbe1_d, [HE, NT], F32)
        w8 = load(w8_d, [HE, NT * EO], F16)
        wdd = load(wdd_d, [HE, NT * HE], F16)
        apt = load(apt_d, [HE, NT * HE], F16)
        ait = load(ait_d, [HE, NT * HE], F16)
        d0c = load(d0_d, [HE, NT], F32)
        wn0 = load(wn0_d, [H + EO, NT * H], F16)
        bn0 = load(bn0_d, [H, NT], F32)
        wn1 = load(wn1_d, [H, NT * H], F16)
        bn1 = load(bn1_d, [H, NT], F32)

        wlin = const.tile([L, N * L], F16, tag="wlin")
        for k in range(4):
            sl = bass.ts(k, N * L // 4)
            eng = nc.sync if k % 2 == 0 else nc.gpsimd
            eng.dma_start(wlin[:, sl], wlin_d[:, sl])

        # ---- stage 1: h0 = x @ W_lin (transposed, perm'd via A/B split) ----
        h0p = ppsum.tile([128, 2 * NP2], F32, tag="prep")
        for np_ in range(NP2):
            lhsT = wlin[:, np_ * 2 * L:(np_ + 1) * 2 * L]
            nc.tensor.matmul(h0p[:, 2 * np_:2 * np_ + 2], lhsT, xTs[:, :],
                             start=True, stop=True, skip_group_check=True)
        hstA = const.tile([L, 2 * NP2], F16, tag="hstA")  # even nodes
        hstB = const.tile([L, 2 * NP2], F16, tag="hstB")  # odd nodes
        nc.scalar.copy(hstA[:, :], h0p[0:L, :])
        nc.scalar.copy(hstB[:, :], h0p[L:2 * L, :])
        hsvA = hstA[:, :].rearrange("p (n two) -> p two n", two=2)
        hsvB = hstB[:, :].rearrange("p (n two) -> p two n", two=2)

        hT = []  # per-b hidden state [H, N] fp16 (perm'd node order)
        for b in range(BPC):
            htp = ppsum.tile([H, N], F32, tag="prep")
            nc.tensor.matmul(htp[:, :], win[:, :], blT[:, :],
                             start=True, stop=False, skip_group_check=True)
            nc.tensor.matmul(htp[:, 0:NP2], win[:, :], hsvA[:, b:b + 1, :],
                             start=False, stop=True, skip_group_check=True)
            nc.tensor.matmul(htp[:, NP2:N], win[:, :], hsvB[:, b:b + 1, :],
                             start=False, stop=True, skip_group_check=True)
            ht = perb.tile([H, N], F16, tag=f"hT{b}")
            nc.scalar.activation(ht[:, :], htp[:, :], AF.Identity,
                                 bias=binc[:, 0:1])
            hT.append(ht)

        # ---- rounds ----
        for t in range(NT):
            w8t = w8[:, bass.ts(t, EO)]
            wddt = wdd[:, bass.ts(t, HE)]
            aptt = apt[:, bass.ts(t, HE)]
            aitt = ait[:, bass.ts(t, HE)]

            stage = {}
            for b in range(BPC):
                ht = hT[b]
                # receivers: bF16 = rcv + be1 (fp16), then
                # delta = G*(We2^T b + be2) = Ap*b + d0, bdelta = b + delta
                rcvp = ppsum.tile([HE, N], F32, tag="prep")
                nc.tensor.matmul(rcvp[:, :], we1a[:, bass.ts(t, HE)], ht[:, :],
                                 start=True, stop=True, skip_group_check=True)
                bF16 = perb.tile([HE, N], F16, tag="bF16")
                nc.scalar.activation(bF16[:, :], rcvp[:, :], AF.Identity,
                                     bias=pbe1[:, t:t + 1])
                dlp = ppsum.tile([HE, N], F32, tag="prep")
                nc.tensor.matmul(dlp[:, :], aptt, bF16[:, :],
                                 start=True, stop=True, skip_group_check=True)
                dl32 = perb.tile([HE, N], F32, tag="dl32")
                nc.scalar.activation(dl32[:, :], dlp[:, :], AF.Identity,
                                     bias=d0c[:, t:t + 1])
                bdp = ppsum.tile([HE, N], F32, tag="prep")
                nc.tensor.matmul(bdp[:, :], aitt, bF16[:, :],
                                 start=True, stop=True, skip_group_check=True)
                bd32 = perb.tile([HE, N], F32, tag="bd32")
                nc.scalar.activation(bd32[:, :], bdp[:, :], AF.Identity,
                                     bias=d0c[:, t:t + 1])

                # senders: snd2 = [snd | snd] fp16
                sndp = ppsum.tile([HE, N], F32, tag="prep")
                nc.tensor.matmul(sndp[:, :], we1b[:, bass.ts(t, HE)], ht[:, :],
                                 start=True, stop=True, skip_group_check=True)
                snd2 = perb.tile([HE, 2 * N], F16, tag="snd2")
                nc.scalar.activation(snd2[:, 0:N], sndp[:, :], AF.Copy)
                nc.vector.tensor_copy(snd2[:, N:2 * N], snd2[:, 0:N])

                mr = perb.tile([HE, NP2], F16, tag=f"mr{b}")
                stage[b] = (bd32, dl32, snd2, mr)

            # ---- edge loop: interleave batches per double-bank ----
            for k in range(NDB):
                for b in range(BPC):
                    bd32, dl32, snd2, mr = stage[b]
                    # m' = max(snd + bdelta_r, delta_r) per receiver
                    ars = []
                    for g in range(2):
                        alow = mpool.tile([HE, 512], F16, tag="mlo")
                        ahigh = mpool.tile([HE, 512], F16, tag="mhi")
                        for j in range(2):
                            rl = 4 * k + 2 * g + j
                            rh = 128 + rl
                            cs = slice(j * 256, (j + 1) * 256)
                            nc.vector.tensor_scalar(
                                alow[:, cs], snd2[:, 0:N],
                                bd32[:, rl:rl + 1], dl32[:, rl:rl + 1],
                                ALU.add, ALU.max)
                            nc.vector.tensor_scalar(
                                ahigh[:, cs], snd2[:, 0:N],
                                bd32[:, rh:rh + 1], dl32[:, rh:rh + 1],
                                ALU.add, ALU.max)
                        ars.append((alow, ahigh))

                    pbs = psum.tile([HE, 1024], F32, tag="pb",
                                    name=f"pb_{t}_{b}_{k}")
                    for g in range(2):
                        alow, ahigh = ars[g]
                        cs = slice(g * 512, (g + 1) * 512)
                        nc.tensor.matmul(pbs[:, cs], wddt, snd2[:, :],
                                         start=True, stop=False,
                                         skip_group_check=True)
                        nc.tensor.matmul(pbs[0:EO, cs], w8t, alow[:, :],
                                         start=False, stop=True,
                                         skip_group_check=True)
                        nc.tensor.matmul(pbs[EO:HE, cs], w8t, ahigh[:, :],
                                         start=False, stop=True,
                                         skip_group_check=True)

                    if k % DVE_REDUCE_EVERY == 0:
                        # Prelu full double-bank -> fp16 arena; DVE X-reduce
                        e2 = e2pool.tile([HE, 1024], F16, tag="e2",
                                         name=f"e2_{t}_{b}_{k}")
                        nc.scalar.activation(e2[:, :], pbs[:, :], AF.Prelu,
                                             alpha=0.2)
                        e2v = e2[:, :].rearrange("p (four n) -> p four n",
                                                 four=4)
                        with nc.allow_low_precision("msg sums fp16 ok"):
                            nc.vector.tensor_reduce(
                                mr[:, 4 * k:4 * k + 4], e2v[:, :, :],
                                axis=mybir.AxisListType.X, op=ALU.add)
                    else:
                        # ACT path: in-place Prelu + accumulator sums
                        with nc.allow_low_precision("msg sums fp16 ok"):
                            for c in range(4):
                                cs = slice(c * N, (c + 1) * N)
                                nc.scalar.activation(
                                    pbs[:, cs], pbs[:, cs], AF.Prelu,
                                    alpha=0.2,
                                    accum_out=mr[:, 4 * k + c:4 * k + c + 1])

            # ---- node MLP ----
            for b in range(BPC):
                _, _, _, mr = stage[b]
                ht = hT[b]
                nT = perb.tile([H + EO, N], F16, tag="nT")
                nc.vector.tensor_copy(nT[0:EO, 0:NP2], mr[0:EO, :])
                nc.vector.tensor_copy(nT[0:EO, NP2:N], mr[EO:HE, :])
                nc.scalar.copy(nT[EO:EO + H, :], ht[:, :])

                n1p = ppsum.tile([H, N], F32, tag="prep")
                nc.tensor.matmul(n1p[:, :], wn0[:, bass.ts(t, H)], nT[:, :],
                                 start=True, stop=True, skip_group_check=True)
                a1 = small.tile([H, N], F16, tag="a1")
                nc.scalar.activation(a1[:, :], n1p[:, :], AF.Prelu, alpha=0.2,
                                     bias=bn0[:, t:t + 1])
                n2p = ppsum.tile([H, N], F32, tag="prep")
                nc.tensor.matmul(n2p[:, :], wn1[:, bass.ts(t, H)], a1[:, :],
                                 start=True, stop=True, skip_group_check=True)
                if t < NT - 1:
                    ht2 = perb.tile([H, N], F16, tag=f"hT{b}")
                    nc.scalar.activation(ht2[:, :], n2p[:, :], AF.Prelu,
                                         alpha=0.2, bias=bn1[:, t:t + 1])
                    hT[b] = ht2
                else:
                    hfin = small.tile([H, N], F32, tag="hfin")
                    nc.scalar.activation(hfin[:, :], n2p[:, :], AF.Prelu,
                                         alpha=0.2, bias=bn1[:, t:t + 1])
                    outT = small.tile([H, N], F32, tag="outT")
                    nc.scalar.activation(outT[:, :], hfin[:, :], AF.Tanh)
                    nc.sync.dma_start(out_d[b, :, :], outT[:, :])

    nc.compile()
    return nc


def _prepare_in_maps(inputs):
    f32 = lambda a: np.ascontiguousarray(np.asarray(a), dtype=np.float32)
    f16c = lambda a: np.ascontiguousarray(np.asarray(a, dtype=np.float32)
                                          .astype(np.float16))
    x = f32(inputs["x"])
    W_lin = f32(inputs["W_lin"])
    b_lin = f32(inputs["b_lin"])
    W_in = f32(inputs["W_in"])
    b_in = f32(inputs["b_in"])
    We1 = f32(inputs["We1"])
    be1 = f32(inputs["be1"])
    We2 = f32(inputs["We2"])
    be2 = f32(inputs["be2"])
    Wn0 = f32(inputs["Wn0"])
    bn0 = f32(inputs["bn0"])
    Wn1 = f32(inputs["Wn1"])
    bn1 = f32(inputs["bn1"])

    blT = b_lin.reshape(N, L).T                  # [L, N]
    blT_perm = np.ascontiguousarray(blT[:, PERM])
    # delta fold: G = 1.25*We2*inv(We2^T We2) so that 0.8*We2^T G = I
    ApTs, AITs, d0s = [], [], []
    for t in range(NT):
        G = 1.25 * We2[t] @ np.linalg.inv(We2[t].T @ We2[t])
        Ap = 0.2 * G @ We2[t].T               # [HE, HE]
        ApTs.append(Ap.T)
        AITs.append((Ap + np.eye(HE, dtype=np.float32)).T)
        d0s.append(G @ be2[t])
    ApT = np.concatenate(ApTs, axis=1)
    AIT = np.concatenate(AITs, axis=1)
    shared = {
        "Wlin16": f16c(W_lin),
        "blT16": f16c(blT_perm),
        "Win16": f16c(W_in),
        "binc": np.ascontiguousarray(b_in[:, None]),
        "We1a16": f16c(We1[:, :H, :].transpose(1, 0, 2).reshape(H, NT * HE)),
        "We1b16": f16c(We1[:, H:, :].transpose(1, 0, 2).reshape(H, NT * HE)),
        "pbe1": np.ascontiguousarray(be1.T),
        "w8c16": f16c(0.8 * We2.transpose(1, 0, 2).reshape(HE, NT * EO)),
        "wdd16": f16c(np.concatenate(
            [np.concatenate([0.2 * We2[t], 0.2 * We2[t]], axis=1)
             for t in range(NT)], axis=1)),
        "ApT16": f16c(ApT),
        "AIT16": f16c(AIT),
        "d0c": np.ascontiguousarray(np.stack(d0s, axis=1)),
        # n is assembled as [m; h] on-chip, so reorder Wn0's input rows
        "Wn0c16": f16c(np.concatenate([Wn0[:, H:, :], Wn0[:, :H, :]], axis=1)
                       .transpose(1, 0, 2).reshape(H + EO, NT * H)),
        "bn0c": np.ascontiguousarray(bn0.T),
        "Wn1c16": f16c(Wn1.transpose(1, 0, 2).reshape(H, NT * H)),
        "bn1c": np.ascontiguousarray(bn1.T),
    }
    in_maps = []
    for c in range(NCORES):
        m = dict(shared)
        m["xT16"] = f16c(x[c * BPC:(c + 1) * BPC, :].T)
        in_maps.append(m)
    return in_maps


def _run(inputs, trace=False):
    from concourse import bass_utils

    if "nc" not in _CACHE:
        _CACHE["nc"] = _build_nc()
    nc = _CACHE["nc"]
    in_maps = _prepare_in_maps(inputs)
    res = bass_utils.run_bass_kernel_spmd(
        nc, in_maps, core_ids=list(range(NCORES)), trace=trace)
    outs = np.concatenate([r["out"] for r in res.results], axis=0)  # [B,H,N]
    out = outs[:, :, INVPERM]            # undo node permutation
    out = np.ascontiguousarray(out.transpose(0, 2, 1))  # [B, N, H]
    return out.astype(np.float32), res


def kernel(**inputs):
    out, _ = _run(inputs, trace=False)
    return out
